# revision 4
# baseline (speedup 1.0000x reference)
"""GAT layer kernel for Trainium2 (8 NeuronCores, batch-parallel).

Math: reference computes, per batch b,
    h     = x @ W                                  (N, F)
    e_ij  = (h@a1)_i + (h@a2)_j   masked by adj_sl = max(adj, I)
    alpha = softmax_j(e)
    out   = alpha @ h + bias

Since the row term (h@a1)_i is constant along the softmax axis it cancels,
so with w_j = exp((h@a2)_j):
    out_i = (sum_j adjsl_ij * w_j * h_j) / (sum_j adjsl_ij * w_j) + bias
which is a single (N x N) @ (N x (1+F)) matmul against V = [w | w*h]:
    P = adj_sl @ V ;  Z = P[:,0] ; out = P[:,1:]/Z + bias

Sharding: one batch element per core (B == n_cores == 8), no collectives.
Per core the only big traffic is adj[b] (16.8 MB) -> memory-bound.

The PE contracts over the partition axis, so adj tiles need j (the
contracted index) on partitions: each natural [128i x 128j] tile is
PE-transposed (identity matmul) into PSUM, copied back to SBUF (DVE/ACT
alternating), then used as the moving operand of the main matmul with
V[J] as the stationary operand, accumulating numT[65, 512] per i-macro.
"""

import numpy as np

B, N, FIN, F = 8, 2048, 128, 64
P = 128
NT = N // P          # 16 j-blocks (and n-tiles)
IM = 4               # i-macro count
IMW = N // IM        # 512 rows per i-macro
SUB = IMW // P       # 4 i-blocks per macro

_CACHE: dict = {}


def _build(adj_bf16: bool):
    from contextlib import ExitStack

    import concourse.bass as bass
    import concourse.tile as tile
    from concourse import bacc, mybir
    from concourse.masks import make_identity

    f32 = mybir.dt.float32
    f32r = mybir.dt.float32r
    bf16 = mybir.dt.bfloat16
    adj_dt = bf16 if adj_bf16 else f32
    FP = F + 1  # 65

    nc = bacc.Bacc("TRN2", target_bir_lowering=False, debug=False, num_devices=B)
    x_d = nc.dram_tensor("x", [N, FIN], f32, kind="ExternalInput").ap()
    adj_d = nc.dram_tensor("adj", [N, N], f32, kind="ExternalInput").ap()
    W_d = nc.dram_tensor("W", [FIN, F], f32, kind="ExternalInput").ap()
    a_d = nc.dram_tensor("a", [2 * F, 1], f32, kind="ExternalInput").ap()
    bias_d = nc.dram_tensor("bias", [F], f32, kind="ExternalInput").ap()
    out_d = nc.dram_tensor("out", [N, F], f32, kind="ExternalOutput").ap()

    with tile.TileContext(nc) as tc, ExitStack() as ctx:
        const = ctx.enter_context(tc.tile_pool(name="const", bufs=1))
        work = ctx.enter_context(tc.tile_pool(name="work", bufs=3))
        adjpool = ctx.enter_context(tc.tile_pool(name="adjc", bufs=2))
        adjT_pool = ctx.enter_context(tc.tile_pool(name="adjT", bufs=3))

        ident = const.tile([P, P], f32)
        make_identity(nc, ident)
        if adj_bf16:
            ident_a = const.tile([P, P], bf16)
            make_identity(nc, ident_a)
        else:
            ident_a = ident

        W_sb = const.tile([FIN, F], f32)
        nc.sync.dma_start(W_sb, W_d)
        a2_sb = const.tile([F, 1], f32)
        nc.sync.dma_start(a2_sb, a_d[F : 2 * F, :])
        bias_row = const.tile([1, F], f32)
        nc.sync.dma_start(bias_row, bias_d[None, :])
        ones_sb = const.tile([1, P], f32)
        nc.vector.memset(ones_sb, 1.0)
        bias_bc = const.tile([P, F], f32)
        Wt = const.tile([F, FIN], f32)
        W_aug = const.tile([FIN, FP], f32)
        Vh = const.tile([P, NT, FP], adj_dt if adj_bf16 else f32r)

        # setup-phase PSUM (closed before the main loop pools open)
        with tc.tile_pool(name="psetup", bufs=2, space="PSUM") as pset:
            ps_b = pset.tile([P, P], f32, tag="ph", name="ps_b")[:, :F]
            nc.tensor.matmul(ps_b, lhsT=ones_sb, rhs=bias_row, start=True, stop=True)
            nc.vector.tensor_copy(bias_bc, ps_b)

            ps_w = pset.tile([P, P], f32, tag="ph", name="ps_w")[:F, :]
            nc.tensor.transpose(ps_w, W_sb, ident)
            nc.vector.tensor_copy(Wt, ps_w)

            ps_wa = pset.tile([P, P], f32, tag="ph", name="ps_wa")[:, :1]
            nc.tensor.matmul(ps_wa, lhsT=Wt, rhs=a2_sb, start=True, stop=True)
            nc.vector.tensor_copy(W_aug[:, 0:F], W_sb)
            nc.vector.tensor_copy(W_aug[:, F : F + 1], ps_wa)

            # h_aug = x @ [W | W@a2]; V[:, j, 0] = w = exp(s2), V[:, j, 1:] = w*h
            for nt in range(NT):
                x_t = work.tile([P, FIN], f32, tag="xt")
                nc.sync.dma_start(x_t, x_d[nt * P : (nt + 1) * P, :])
                ps_x = pset.tile([P, P], f32, tag="ph")
                nc.tensor.transpose(ps_x, x_t, ident)
                xT = work.tile([P, P], f32, tag="xTt")
                nc.vector.tensor_copy(xT, ps_x)
                ps_h = pset.tile([P, P], f32, tag="ph", name="ps_h")[:, :FP]
                nc.tensor.matmul(ps_h, lhsT=xT, rhs=W_aug, start=True, stop=True)
                w_t = work.tile([P, 1], f32, tag="wt")
                nc.scalar.activation(
                    w_t, ps_h[:, F : F + 1], mybir.ActivationFunctionType.Exp
                )
                nc.vector.tensor_scalar_mul(Vh[:, nt, 1:FP], ps_h[:, 0:F], w_t)
                nc.vector.tensor_copy(Vh[:, nt, 0:1], w_t)

        psum_t = ctx.enter_context(tc.tile_pool(name="pst", bufs=2, space="PSUM"))
        psum_a = ctx.enter_context(tc.tile_pool(name="psa", bufs=2, space="PSUM"))
        psum_o = ctx.enter_context(tc.tile_pool(name="pso", bufs=2, space="PSUM"))

        for I in range(IM):
            chunk = adjpool.tile([P, SUB, N], adj_dt, tag="chunk")
            src = adj_d[I * IMW : (I + 1) * IMW, :].rearrange("(a p) j -> p a j", p=P)
            if adj_bf16:
                nc.gpsimd.dma_start(chunk, src)  # casts f32 -> bf16 inline
            else:
                nc.sync.dma_start(chunk, src)

            psa = psum_a.tile([FP, IMW], f32, tag="acc")
            for J in range(NT):
                pst = psum_t.tile([P, IMW], adj_dt, tag="tr")
                for t in range(SUB):
                    nc.tensor.transpose(
                        pst[:, t * P : (t + 1) * P],
                        chunk[:, t, J * P : (J + 1) * P],
                        ident_a,
                    )
                adjT = adjT_pool.tile([P, IMW], adj_dt if adj_bf16 else f32r, tag="adjT")
                if J % 2 == 0:
                    nc.vector.tensor_copy(adjT, pst)
                else:
                    nc.scalar.copy(adjT, pst)
                if I * SUB <= J < (I + 1) * SUB:
                    # diagonal block: adj_sl = max(adj, I) for self-loops
                    t0 = (J - I * SUB) * P
                    nc.vector.tensor_max(
                        adjT[:, t0 : t0 + P], adjT[:, t0 : t0 + P], ident_a
                    )
                nc.tensor.matmul(
                    psa, lhsT=Vh[:, J, :], rhs=adjT[:], start=(J == 0), stop=(J == NT - 1)
                )

            numT = work.tile([FP, IMW], f32, tag="numT")
            nc.vector.tensor_copy(numT, psa)
            for t in range(SUB):
                pso = psum_o.tile([P, FP], f32, tag="o")
                nc.tensor.transpose(
                    pso, numT[:, t * P : (t + 1) * P], ident[:FP, :FP]
                )
                recip = work.tile([P, 1], f32, tag="rc")
                nc.vector.reciprocal(recip, pso[:, 0:1])
                o_sb = work.tile([P, F], f32, tag="osb")
                nc.vector.tensor_scalar_mul(o_sb, pso[:, 1:FP], recip)
                nc.vector.tensor_add(o_sb, o_sb, bias_bc)
                ib = I * SUB + t
                nc.sync.dma_start(out_d[ib * P : (ib + 1) * P, :], o_sb)

    nc.compile()
    return nc


def _get_nc(adj_bf16: bool = False):
    key = ("nc", adj_bf16)
    if key not in _CACHE:
        _CACHE[key] = _build(adj_bf16)
    return _CACHE[key]


def kernel(x, adj, W, a, bias, adj_bf16: bool = False):
    from concourse import bass_utils

    nc = _get_nc(adj_bf16)
    in_maps = [
        {
            "x": np.ascontiguousarray(x[b], dtype=np.float32),
            "adj": np.ascontiguousarray(adj[b], dtype=np.float32),
            "W": np.ascontiguousarray(W, dtype=np.float32),
            "a": np.ascontiguousarray(a, dtype=np.float32),
            "bias": np.ascontiguousarray(bias, dtype=np.float32),
        }
        for b in range(B)
    ]
    res = bass_utils.run_bass_kernel_spmd(nc, in_maps, core_ids=list(range(B)))
    return np.stack([res.results[b]["out"] for b in range(B)]).astype(np.float32)


# revision 5
# speedup vs baseline: 1.2461x; 1.2461x over previous
"""GAT layer kernel for Trainium2 (8 NeuronCores, batch-parallel).

Math: reference computes, per batch b,
    h     = x @ W                                  (N, F)
    e_ij  = (h@a1)_i + (h@a2)_j   masked by adj_sl = max(adj, I)
    alpha = softmax_j(e)
    out   = alpha @ h + bias

Since the row term (h@a1)_i is constant along the softmax axis it cancels,
so with w_j = exp((h@a2)_j):
    out_i = (sum_j adjsl_ij * w_j * h_j) / (sum_j adjsl_ij * w_j) + bias
which is a single (N x N) @ (N x (1+F)) matmul against V = [w | w*h]:
    P = adj_sl @ V ;  Z = P[:,0] ; out = P[:,1:]/Z + bias

Sharding: one batch element per core (B == n_cores == 8), no collectives.
Per core the only big traffic is adj[b] (16.8 MB) -> memory-bound.

The PE contracts over the partition axis, so adj tiles need j (the
contracted index) on partitions: each natural [128i x 128j] tile is
PE-transposed (identity matmul) into PSUM, copied back to SBUF (DVE/ACT
alternating), then used as the moving operand of the main matmul with
V[J] as the stationary operand, accumulating numT[65, 512] per i-macro.
The J loop is software-pipelined (matmul lags the transposes by MM_LAG)
so the PE never stalls on the PSUM->SBUF copy latency.
"""

import numpy as np

B, N, FIN, F = 8, 2048, 128, 64
P = 128
NT = N // P          # 16 j-blocks (and n-tiles)
IM = 4               # i-macro count
IMW = N // IM        # 512 rows per i-macro
SUB = IMW // P       # 4 i-blocks per macro
FP = F + 1           # 65
MM_LAG = 2           # J-loop software pipeline depth

_CACHE: dict = {}


def _build(adj_bf16: bool):
    from contextlib import ExitStack

    import concourse.tile as tile
    from concourse import bacc, mybir
    from concourse.masks import make_identity

    f32 = mybir.dt.float32
    f32r = mybir.dt.float32r
    bf16 = mybir.dt.bfloat16
    adj_dt = bf16 if adj_bf16 else f32
    mm_dt = bf16 if adj_bf16 else f32r

    nc = bacc.Bacc("TRN2", target_bir_lowering=False, debug=False, num_devices=B)
    x_d = nc.dram_tensor("x", [N, FIN], f32, kind="ExternalInput").ap()
    adj_d = nc.dram_tensor("adj", [N, N], f32, kind="ExternalInput").ap()
    W_d = nc.dram_tensor("W", [FIN, F], f32, kind="ExternalInput").ap()
    a_d = nc.dram_tensor("a", [2 * F, 1], f32, kind="ExternalInput").ap()
    bias_d = nc.dram_tensor("bias", [F], f32, kind="ExternalInput").ap()
    out_d = nc.dram_tensor("out", [N, F], f32, kind="ExternalOutput").ap()

    with tile.TileContext(nc) as tc, ExitStack() as ctx:
        const = ctx.enter_context(tc.tile_pool(name="const", bufs=1))
        work = ctx.enter_context(tc.tile_pool(name="work", bufs=3))
        xpool = ctx.enter_context(tc.tile_pool(name="xp", bufs=NT))
        adjpool = ctx.enter_context(tc.tile_pool(name="adjc", bufs=2))
        adjT_pool = ctx.enter_context(tc.tile_pool(name="adjT", bufs=MM_LAG + 2))

        ident = const.tile([P, P], f32)
        make_identity(nc, ident)
        if adj_bf16:
            ident_a = const.tile([P, P], bf16)
            make_identity(nc, ident_a)
        else:
            ident_a = ident

        W_sb = const.tile([FIN, F], f32)
        nc.sync.dma_start(W_sb, W_d)
        a2_sb = const.tile([F, 1], f32)
        nc.sync.dma_start(a2_sb, a_d[F : 2 * F, :])
        bias_row = const.tile([1, F], f32)
        nc.sync.dma_start(bias_row, bias_d[None, :])
        ones_sb = const.tile([1, P], f32)
        nc.vector.memset(ones_sb, 1.0)
        bias_bc = const.tile([P, F], f32)
        Wt = const.tile([F, FIN], f32)
        W_aug = const.tile([FIN, FP], f32)
        Vh = const.tile([P, NT, FP], mm_dt)

        # ---- setup phase: W_aug = [W | W@a2], bias broadcast ----
        with tc.tile_pool(name="psetup", bufs=3, space="PSUM") as pset:
            ps_b = pset.tile([P, P], f32, tag="ph", name="ps_b")[:, :F]
            nc.tensor.matmul(ps_b, lhsT=ones_sb, rhs=bias_row, start=True, stop=True)
            nc.vector.tensor_copy(bias_bc, ps_b)

            ps_w = pset.tile([P, P], f32, tag="ph", name="ps_w")[:F, :]
            nc.tensor.transpose(ps_w, W_sb, ident)
            nc.vector.tensor_copy(Wt, ps_w)

            ps_wa = pset.tile([P, P], f32, tag="ph", name="ps_wa")[:, :1]
            nc.tensor.matmul(ps_wa, lhsT=Wt, rhs=a2_sb, start=True, stop=True)
            nc.vector.tensor_copy(W_aug[:, 0:F], W_sb)
            nc.vector.tensor_copy(W_aug[:, F : F + 1], ps_wa)

            # ---- h stage, pipelined: all x transposes first, then matmuls ----
            # h_aug = x @ [W | W@a2]; V[:, j, 0] = w = exp(s2), V[:, j, 1:] = w*h
            xTs = []
            for nt in range(NT):
                x_t = xpool.tile([P, FIN], f32, tag="xt", name="x_t")
                nc.sync.dma_start(x_t, x_d[nt * P : (nt + 1) * P, :])
                ps_x = pset.tile([P, P], f32, tag="ph", name="ps_x")
                nc.tensor.transpose(ps_x, x_t, ident)
                xT = xpool.tile([P, P], f32, tag="xTt", name="xT")
                nc.vector.tensor_copy(xT, ps_x)
                xTs.append(xT)
            for nt in range(NT):
                ps_h = pset.tile([P, P], f32, tag="psh", name="ps_h")[:, :FP]
                nc.tensor.matmul(ps_h, lhsT=xTs[nt], rhs=W_aug, start=True, stop=True)
                w_t = work.tile([P, 1], f32, tag="wt")
                nc.scalar.activation(
                    w_t, ps_h[:, F : F + 1], mybir.ActivationFunctionType.Exp
                )
                nc.vector.tensor_scalar_mul(Vh[:, nt, 1:FP], ps_h[:, 0:F], w_t)
                nc.vector.tensor_copy(Vh[:, nt, 0:1], w_t)

        psum_t = ctx.enter_context(
            tc.tile_pool(name="pst", bufs=MM_LAG + 1, space="PSUM")
        )
        psum_a = ctx.enter_context(tc.tile_pool(name="psa", bufs=2, space="PSUM"))
        psum_o = ctx.enter_context(tc.tile_pool(name="pso", bufs=2, space="PSUM"))

        # ---- main loop: numT[I] = (adj_sl @ V).T for each i-macro ----
        for I in range(IM):
            chunk = adjpool.tile([P, SUB, N], adj_dt, tag="chunk", name="chunk")
            src = adj_d[I * IMW : (I + 1) * IMW, :].rearrange("(a p) j -> p a j", p=P)
            if adj_bf16:
                nc.gpsimd.dma_start(chunk, src)  # casts f32 -> bf16 inline
            else:
                nc.sync.dma_start(chunk, src)

            psa = psum_a.tile([FP, IMW], f32, tag="acc", name="psa")
            pending = []  # software-pipelined matmuls: PE never waits on a copy
            for J in range(NT):
                pst = psum_t.tile([P, IMW], adj_dt, tag="tr", name="pst")
                for t in range(SUB):
                    nc.tensor.transpose(
                        pst[:, t * P : (t + 1) * P],
                        chunk[:, t, J * P : (J + 1) * P],
                        ident_a,
                    )
                adjT = adjT_pool.tile([P, IMW], mm_dt, tag="adjT", name="adjT")
                if J % 2 == 0:
                    nc.vector.tensor_copy(adjT, pst)
                else:
                    nc.scalar.copy(adjT, pst)
                if I * SUB <= J < (I + 1) * SUB:
                    # diagonal block: adj_sl = max(adj, I) for self-loops
                    t0 = (J - I * SUB) * P
                    nc.vector.tensor_max(
                        adjT[:, t0 : t0 + P], adjT[:, t0 : t0 + P], ident_a
                    )
                pending.append((Vh[:, J, :], adjT, J == 0, J == NT - 1))
                if len(pending) > MM_LAG:
                    lhsT, rhs, st, sp = pending.pop(0)
                    nc.tensor.matmul(psa, lhsT=lhsT, rhs=rhs[:], start=st, stop=sp)
            for lhsT, rhs, st, sp in pending:
                nc.tensor.matmul(psa, lhsT=lhsT, rhs=rhs[:], start=st, stop=sp)

            # ---- epilogue: out[i] = num/Z + bias, back in [i, f] layout ----
            numT = work.tile([FP, IMW], f32, tag="numT", name="numT")
            nc.vector.tensor_copy(numT, psa)
            for t in range(SUB):
                pso = psum_o.tile([P, FP], f32, tag="o", name="pso")
                nc.tensor.transpose(pso, numT[:, t * P : (t + 1) * P], ident[:FP, :FP])
                recip = work.tile([P, 1], f32, tag="rc", name="recip")
                nc.vector.reciprocal(recip, pso[:, 0:1])
                o_sb = work.tile([P, F], f32, tag="osb", name="o_sb")
                nc.vector.tensor_scalar_mul(o_sb, pso[:, 1:FP], recip)
                nc.vector.tensor_add(o_sb, o_sb, bias_bc)
                ib = I * SUB + t
                nc.sync.dma_start(out_d[ib * P : (ib + 1) * P, :], o_sb)

    nc.compile()
    return nc


def _get_nc(adj_bf16: bool = False):
    key = ("nc", adj_bf16)
    if key not in _CACHE:
        _CACHE[key] = _build(adj_bf16)
    return _CACHE[key]


def kernel(x, adj, W, a, bias, adj_bf16: bool = False):
    from concourse import bass_utils

    nc = _get_nc(adj_bf16)
    in_maps = [
        {
            "x": np.ascontiguousarray(x[b], dtype=np.float32),
            "adj": np.ascontiguousarray(adj[b], dtype=np.float32),
            "W": np.ascontiguousarray(W, dtype=np.float32),
            "a": np.ascontiguousarray(a, dtype=np.float32),
            "bias": np.ascontiguousarray(bias, dtype=np.float32),
        }
        for b in range(B)
    ]
    res = bass_utils.run_bass_kernel_spmd(nc, in_maps, core_ids=list(range(B)))
    return np.stack([res.results[b]["out"] for b in range(B)]).astype(np.float32)


# revision 8
# speedup vs baseline: 1.3475x; 1.0814x over previous
"""GAT layer kernel for Trainium2 (8 NeuronCores, batch-parallel).

Math: reference computes, per batch b,
    h     = x @ W                                  (N, F)
    e_ij  = (h@a1)_i + (h@a2)_j   masked by adj_sl = max(adj, I)
    alpha = softmax_j(e)
    out   = alpha @ h + bias

Since the row term (h@a1)_i is constant along the softmax axis it cancels,
so with w_j = exp((h@a2)_j):
    out_i = (sum_j adjsl_ij * w_j * h_j) / (sum_j adjsl_ij * w_j) + bias
which is a single (N x N) @ (N x (1+F)) matmul against V = [w | w*h]:
    P = adj_sl @ V ;  Z = P[:,0] ; out = P[:,1:]/Z + bias

Sharding: one batch element per core (B == n_cores == 8), no collectives.
Per core the only big traffic is adj[b] (16.8 MB) -> memory-bound.

The PE contracts over the partition axis, so adj tiles need j (the
contracted index) on partitions: each natural [128i x 128j] tile is
PE-transposed (identity matmul) into PSUM, copied back to SBUF (DVE/ACT
alternating), then used as the moving operand of the main matmul with
V[J] as the stationary operand, accumulating numT[65, 512] per i-macro.
The J loop is software-pipelined (matmul lags the transposes by MM_LAG)
so the PE never stalls on the PSUM->SBUF copy latency.
"""

import numpy as np

B, N, FIN, F = 8, 2048, 128, 64
P = 128
NT = N // P          # 16 j-blocks (and n-tiles)
IM = 4               # i-macro count
IMW = N // IM        # 512 rows per i-macro
SUB = IMW // P       # 4 i-blocks per macro
FP = F + 1           # 65
MM_LAG = 3           # J-loop software pipeline depth

_CACHE: dict = {}


def _build(adj_bf16: bool):
    from contextlib import ExitStack

    import concourse.tile as tile
    from concourse import bacc, mybir
    from concourse.masks import make_identity

    f32 = mybir.dt.float32
    f32r = mybir.dt.float32r
    bf16 = mybir.dt.bfloat16
    adj_dt = bf16 if adj_bf16 else f32r
    mm_dt = bf16 if adj_bf16 else f32r

    nc = bacc.Bacc("TRN2", target_bir_lowering=False, debug=False, num_devices=B)
    x_d = nc.dram_tensor("x", [N, FIN], f32, kind="ExternalInput").ap()
    adj_d = nc.dram_tensor("adj", [N, N], adj_dt, kind="ExternalInput").ap()
    W_d = nc.dram_tensor("W", [FIN, F], f32, kind="ExternalInput").ap()
    a_d = nc.dram_tensor("a", [2 * F, 1], f32, kind="ExternalInput").ap()
    bias_d = nc.dram_tensor("bias", [F], f32, kind="ExternalInput").ap()
    out_d = nc.dram_tensor("out", [N, F], f32, kind="ExternalOutput").ap()

    with tile.TileContext(nc) as tc, ExitStack() as ctx:
        const = ctx.enter_context(tc.tile_pool(name="const", bufs=1))
        work = ctx.enter_context(tc.tile_pool(name="work", bufs=3))
        xpool = ctx.enter_context(tc.tile_pool(name="xp", bufs=NT))
        adjpool = ctx.enter_context(tc.tile_pool(name="adjc", bufs=3))
        adjT_pool = ctx.enter_context(tc.tile_pool(name="adjT", bufs=MM_LAG + 2))

        ident = const.tile([P, P], f32)
        make_identity(nc, ident)
        ident_a = const.tile([P, P], adj_dt)
        if adj_bf16:
            make_identity(nc, ident_a)
        else:
            # memset/affine_select reject f32r; cast-copy from the f32 identity
            nc.vector.tensor_copy(ident_a, ident)

        W_sb = const.tile([FIN, F], f32)
        nc.sync.dma_start(W_sb, W_d)
        a2_sb = const.tile([F, 1], f32)
        nc.sync.dma_start(a2_sb, a_d[F : 2 * F, :])
        bias_row = const.tile([1, F], f32)
        nc.sync.dma_start(bias_row, bias_d[None, :])
        ones_sb = const.tile([1, P], f32)
        nc.vector.memset(ones_sb, 1.0)
        bias_bc = const.tile([P, F], f32)
        Wt = const.tile([F, FIN], f32)
        W_aug = const.tile([FIN, FP], f32)
        Vh = const.tile([P, NT, FP], mm_dt)

        # ---- setup phase: W_aug = [W | W@a2], bias broadcast ----
        with tc.tile_pool(name="psetup", bufs=3, space="PSUM") as pset:
            ps_b = pset.tile([P, P], f32, tag="ph", name="ps_b")[:, :F]
            nc.tensor.matmul(ps_b, lhsT=ones_sb, rhs=bias_row, start=True, stop=True)
            nc.vector.tensor_copy(bias_bc, ps_b)

            ps_w = pset.tile([P, P], f32, tag="ph", name="ps_w")[:F, :]
            nc.tensor.transpose(ps_w, W_sb, ident)
            nc.vector.tensor_copy(Wt, ps_w)

            ps_wa = pset.tile([P, P], f32, tag="ph", name="ps_wa")[:, :1]
            nc.tensor.matmul(ps_wa, lhsT=Wt, rhs=a2_sb, start=True, stop=True)
            nc.vector.tensor_copy(W_aug[:, 0:F], W_sb)
            nc.vector.tensor_copy(W_aug[:, F : F + 1], ps_wa)

            # ---- h stage, pipelined: all x transposes first, then matmuls ----
            # h_aug = x @ [W | W@a2]; V[:, j, 0] = w = exp(s2), V[:, j, 1:] = w*h
            xTs = []
            for nt in range(NT):
                x_t = xpool.tile([P, FIN], f32, tag="xt", name="x_t")
                nc.sync.dma_start(x_t, x_d[nt * P : (nt + 1) * P, :])
                ps_x = pset.tile([P, P], f32, tag="ph", name="ps_x")
                nc.tensor.transpose(ps_x, x_t, ident)
                xT = xpool.tile([P, P], f32, tag="xTt", name="xT")
                nc.scalar.copy(xT, ps_x)
                xTs.append(xT)
            for nt in range(NT):
                ps_h = pset.tile([P, P], f32, tag="psh", name="ps_h")[:, :FP]
                nc.tensor.matmul(ps_h, lhsT=xTs[nt], rhs=W_aug, start=True, stop=True)
                w_t = work.tile([P, 1], f32, tag="wt")
                nc.scalar.activation(
                    w_t, ps_h[:, F : F + 1], mybir.ActivationFunctionType.Exp
                )
                nc.vector.tensor_scalar_mul(Vh[:, nt, 1:FP], ps_h[:, 0:F], w_t)
                nc.vector.tensor_copy(Vh[:, nt, 0:1], w_t)

        psum_t = ctx.enter_context(
            tc.tile_pool(name="pst", bufs=MM_LAG + 1, space="PSUM")
        )
        psum_a = ctx.enter_context(tc.tile_pool(name="psa", bufs=2, space="PSUM"))
        psum_o = ctx.enter_context(tc.tile_pool(name="pso", bufs=2, space="PSUM"))

        # ---- main loop: numT[I] = (adj_sl @ V).T for each i-macro ----
        for I in range(IM):
            chunk = adjpool.tile([P, SUB, N], adj_dt, tag="chunk", name="chunk")
            src = adj_d[I * IMW : (I + 1) * IMW, :].rearrange("(a p) j -> p a j", p=P)
            if adj_bf16:
                nc.gpsimd.dma_start(chunk, src)  # casts f32 -> bf16 inline
            else:
                nc.sync.dma_start(chunk, src)

            psa = psum_a.tile([FP, IMW], f32, tag="acc", name="psa")
            pending = []  # software-pipelined matmuls: PE never waits on a copy
            for J in range(NT):
                pst = psum_t.tile([P, IMW], adj_dt, tag="tr", name="pst")
                for t in range(SUB):
                    nc.tensor.transpose(
                        pst[:, t * P : (t + 1) * P],
                        chunk[:, t, J * P : (J + 1) * P],
                        ident_a,
                    )
                adjT = adjT_pool.tile([P, IMW], mm_dt, tag="adjT", name="adjT")
                if J % 2 == 0:
                    nc.vector.tensor_copy(adjT, pst)
                else:
                    nc.scalar.copy(adjT, pst)
                if I * SUB <= J < (I + 1) * SUB:
                    # diagonal block: adj_sl = max(adj, I) for self-loops
                    t0 = (J - I * SUB) * P
                    nc.vector.tensor_max(
                        adjT[:, t0 : t0 + P], adjT[:, t0 : t0 + P], ident_a
                    )
                pending.append((Vh[:, J, :], adjT, J == 0, J == NT - 1))
                if len(pending) > MM_LAG:
                    lhsT, rhs, st, sp = pending.pop(0)
                    nc.tensor.matmul(psa, lhsT=lhsT, rhs=rhs[:], start=st, stop=sp)
            for lhsT, rhs, st, sp in pending:
                nc.tensor.matmul(psa, lhsT=lhsT, rhs=rhs[:], start=st, stop=sp)

            # ---- epilogue: out[i] = num/Z + bias, back in [i, f] layout ----
            numT = work.tile([FP, IMW], f32, tag="numT", name="numT")
            nc.scalar.copy(numT, psa)
            for t in range(SUB):
                pso = psum_o.tile([P, FP], f32, tag="o", name="pso")
                nc.tensor.transpose(pso, numT[:, t * P : (t + 1) * P], ident[:FP, :FP])
                recip = work.tile([P, 1], f32, tag="rc", name="recip")
                nc.vector.reciprocal(recip, pso[:, 0:1])
                o_sb = work.tile([P, F], f32, tag="osb", name="o_sb")
                nc.vector.scalar_tensor_tensor(
                    o_sb, pso[:, 1:FP], recip, bias_bc,
                    mybir.AluOpType.mult, mybir.AluOpType.add,
                )
                ib = I * SUB + t
                nc.sync.dma_start(out_d[ib * P : (ib + 1) * P, :], o_sb)

    nc.compile()
    return nc


def _get_nc(adj_bf16: bool = False):
    key = ("nc", adj_bf16)
    if key not in _CACHE:
        _CACHE[key] = _build(adj_bf16)
    return _CACHE[key]


def kernel(x, adj, W, a, bias, adj_bf16: bool = False):
    from concourse import bass_utils

    nc = _get_nc(adj_bf16)
    in_maps = [
        {
            "x": np.ascontiguousarray(x[b], dtype=np.float32),
            "adj": np.ascontiguousarray(adj[b], dtype=np.float32),
            "W": np.ascontiguousarray(W, dtype=np.float32),
            "a": np.ascontiguousarray(a, dtype=np.float32),
            "bias": np.ascontiguousarray(bias, dtype=np.float32),
        }
        for b in range(B)
    ]
    res = bass_utils.run_bass_kernel_spmd(nc, in_maps, core_ids=list(range(B)))
    return np.stack([res.results[b]["out"] for b in range(B)]).astype(np.float32)


# revision 10
# speedup vs baseline: 1.6480x; 1.2230x over previous
"""GAT layer kernel for Trainium2 (8 NeuronCores, batch-parallel).

Math: reference computes, per batch b,
    h     = x @ W                                  (N, F)
    e_ij  = (h@a1)_i + (h@a2)_j   masked by adj_sl = max(adj, I)
    alpha = softmax_j(e)
    out   = alpha @ h + bias

Since the row term (h@a1)_i is constant along the softmax axis it cancels,
so with w_j = exp((h@a2)_j):
    out_i = (sum_j adjsl_ij * w_j * h_j) / (sum_j adjsl_ij * w_j) + bias
which is a single (N x N) @ (N x (1+F)) matmul against V = [w | w*h]:
    P = adj_sl @ V ;  Z = P[:,0] ; out = P[:,1:]/Z + bias

Sharding: one batch element per core (B == n_cores == 8), no collectives.
Per core the only big traffic is adj[b] (16.8 MB) -> memory-bound.

The PE contracts over the partition axis, so adj tiles need j (the
contracted index) on partitions: each natural [128i x 128j] tile is
PE-transposed (identity matmul) into PSUM, copied back to SBUF (DVE/ACT
alternating), then used as the moving operand of the main matmul with
V[J] as the stationary operand, accumulating numT[65, 512] per i-macro.
The J loop is software-pipelined (matmul lags the transposes by MM_LAG)
so the PE never stalls on the PSUM->SBUF copy latency.
"""

import numpy as np

B, N, FIN, F = 8, 2048, 128, 64
P = 128
NT = N // P          # 16 j-blocks (and n-tiles)
IM = 4               # i-macro count
IMW = N // IM        # 512 rows per i-macro
SUB = IMW // P       # 4 i-blocks per macro
FP = F + 1           # 65
MM_LAG = 3           # J-loop software pipeline depth

_CACHE: dict = {}


def _build(adj_bf16: bool):
    from contextlib import ExitStack

    import concourse.tile as tile
    from concourse import bacc, mybir
    from concourse.masks import make_identity

    f32 = mybir.dt.float32
    f32r = mybir.dt.float32r
    bf16 = mybir.dt.bfloat16
    adj_dt = bf16 if adj_bf16 else f32r
    mm_dt = bf16 if adj_bf16 else f32r

    nc = bacc.Bacc("TRN2", target_bir_lowering=False, debug=False, num_devices=B)
    x_d = nc.dram_tensor("x", [N, FIN], f32, kind="ExternalInput").ap()
    adj_d = nc.dram_tensor("adj", [N, N], adj_dt, kind="ExternalInput").ap()
    W_d = nc.dram_tensor("W", [FIN, F], f32, kind="ExternalInput").ap()
    a_d = nc.dram_tensor("a", [2 * F, 1], f32, kind="ExternalInput").ap()
    bias_d = nc.dram_tensor("bias", [F], f32, kind="ExternalInput").ap()
    out_d = nc.dram_tensor("out", [N, F], f32, kind="ExternalOutput").ap()

    with tile.TileContext(nc) as tc, ExitStack() as ctx:
        const = ctx.enter_context(tc.tile_pool(name="const", bufs=1))
        work = ctx.enter_context(tc.tile_pool(name="work", bufs=3))
        xpool = ctx.enter_context(tc.tile_pool(name="xp", bufs=NT))
        xallpool = ctx.enter_context(tc.tile_pool(name="xap", bufs=1))
        adjpool = ctx.enter_context(tc.tile_pool(name="adjc", bufs=3 * 4))
        adjT_pool = ctx.enter_context(tc.tile_pool(name="adjT", bufs=MM_LAG + 2))

        ident = const.tile([P, P], f32)
        make_identity(nc, ident)
        ident_a = const.tile([P, P], adj_dt)
        if adj_bf16:
            make_identity(nc, ident_a)
        else:
            # memset/affine_select reject f32r; cast-copy from the f32 identity
            nc.vector.tensor_copy(ident_a, ident)

        W_sb = const.tile([FIN, F], f32)
        nc.sync.dma_start(W_sb, W_d)
        a2_sb = const.tile([F, 1], f32)
        nc.sync.dma_start(a2_sb, a_d[F : 2 * F, :])
        bias_row = const.tile([1, F], f32)
        nc.sync.dma_start(bias_row, bias_d[None, :])
        ones_sb = const.tile([1, P], f32)
        nc.vector.memset(ones_sb, 1.0)
        bias_bc = const.tile([P, F], f32)
        Wt = const.tile([F, FIN], f32)
        W_aug = const.tile([FIN, FP], f32)
        Vh = const.tile([P, NT, FP], mm_dt)

        # ---- setup phase: W_aug = [W | W@a2], bias broadcast ----
        with tc.tile_pool(name="psetup", bufs=3, space="PSUM") as pset:
            ps_b = pset.tile([P, P], f32, tag="ph", name="ps_b")[:, :F]
            nc.tensor.matmul(ps_b, lhsT=ones_sb, rhs=bias_row, start=True, stop=True)
            nc.vector.tensor_copy(bias_bc, ps_b)

            ps_w = pset.tile([P, P], f32, tag="ph", name="ps_w")[:F, :]
            nc.tensor.transpose(ps_w, W_sb, ident)
            nc.vector.tensor_copy(Wt, ps_w)

            ps_wa = pset.tile([P, P], f32, tag="ph", name="ps_wa")[:, :1]
            nc.tensor.matmul(ps_wa, lhsT=Wt, rhs=a2_sb, start=True, stop=True)
            nc.vector.tensor_copy(W_aug[:, 0:F], W_sb)
            nc.vector.tensor_copy(W_aug[:, F : F + 1], ps_wa)

            # ---- h stage, pipelined: all x transposes first, then matmuls ----
            # h_aug = x @ [W | W@a2]; V[:, j, 0] = w = exp(s2), V[:, j, 1:] = w*h
            x_all = xallpool.tile([P, NT, FIN], f32, tag="xall", name="x_all")
            nc.sync.dma_start(x_all, x_d.rearrange("(o p) c -> p o c", p=P))
            xTs = []
            for nt in range(NT):
                ps_x = pset.tile([P, P], f32, tag="ph", name="ps_x")
                nc.tensor.transpose(ps_x, x_all[:, nt, :], ident)
                xT = xpool.tile([P, P], f32, tag="xTt", name="xT")
                nc.scalar.copy(xT, ps_x)
                xTs.append(xT)
            for nt in range(NT):
                ps_h = pset.tile([P, P], f32, tag="psh", name="ps_h")[:, :FP]
                nc.tensor.matmul(ps_h, lhsT=xTs[nt], rhs=W_aug, start=True, stop=True)
                w_t = work.tile([P, 1], f32, tag="wt")
                nc.scalar.activation(
                    w_t, ps_h[:, F : F + 1], mybir.ActivationFunctionType.Exp
                )
                nc.vector.tensor_scalar_mul(Vh[:, nt, 1:FP], ps_h[:, 0:F], w_t)
                nc.vector.tensor_copy(Vh[:, nt, 0:1], w_t)

        psum_t = ctx.enter_context(
            tc.tile_pool(name="pst", bufs=MM_LAG + 1, space="PSUM")
        )
        psum_a = ctx.enter_context(tc.tile_pool(name="psa", bufs=2, space="PSUM"))
        psum_o = ctx.enter_context(tc.tile_pool(name="pso", bufs=2, space="PSUM"))

        # ---- main loop: numT[I] = (adj_sl @ V).T for each i-macro ----
        # adj arrives as j-quarters [P, SUB, IMW] so compute streams with DMA
        NQ = NT // SUB  # 4 j-quarters per i-macro

        def load_quarters(I):
            qs = []
            for q in range(NQ):
                cq = adjpool.tile([P, SUB, IMW], adj_dt, tag="chunk", name="cq")
                src = adj_d[
                    I * IMW : (I + 1) * IMW, q * IMW : (q + 1) * IMW
                ].rearrange("(a p) j -> p a j", p=P)
                if adj_bf16:
                    nc.gpsimd.dma_start(cq, src)  # casts f32 -> bf16 inline
                else:
                    nc.sync.dma_start(cq, src)
                qs.append(cq)
            return qs

        quarters = {0: load_quarters(0)}
        for I in range(IM):
            if I + 1 < IM:
                quarters[I + 1] = load_quarters(I + 1)
            qs = quarters.pop(I)

            psa = psum_a.tile([FP, IMW], f32, tag="acc", name="psa")
            pending = []  # software-pipelined matmuls: PE never waits on a copy
            for J in range(NT):
                cq = qs[J // SUB]
                jc = (J % SUB) * P
                pst = psum_t.tile([P, IMW], adj_dt, tag="tr", name="pst")
                for t in range(SUB):
                    nc.tensor.transpose(
                        pst[:, t * P : (t + 1) * P],
                        cq[:, t, jc : jc + P],
                        ident_a,
                    )
                adjT = adjT_pool.tile([P, IMW], mm_dt, tag="adjT", name="adjT")
                if J % 2 == 0:
                    nc.vector.tensor_copy(adjT, pst)
                else:
                    nc.scalar.copy(adjT, pst)
                if I * SUB <= J < (I + 1) * SUB:
                    # diagonal block: adj_sl = max(adj, I) for self-loops
                    t0 = (J - I * SUB) * P
                    nc.vector.tensor_max(
                        adjT[:, t0 : t0 + P], adjT[:, t0 : t0 + P], ident_a
                    )
                pending.append((Vh[:, J, :], adjT, J == 0, J == NT - 1))
                if len(pending) > MM_LAG:
                    lhsT, rhs, st, sp = pending.pop(0)
                    nc.tensor.matmul(psa, lhsT=lhsT, rhs=rhs[:], start=st, stop=sp)
            for lhsT, rhs, st, sp in pending:
                nc.tensor.matmul(psa, lhsT=lhsT, rhs=rhs[:], start=st, stop=sp)

            # ---- epilogue: out[i] = num/Z + bias, back in [i, f] layout ----
            numT = work.tile([FP, IMW], f32, tag="numT", name="numT")
            nc.scalar.copy(numT, psa)
            o_sb = work.tile([P, SUB, F], f32, tag="osb", name="o_sb")
            for t in range(SUB):
                pso = psum_o.tile([P, FP], f32, tag="o", name="pso")
                nc.tensor.transpose(pso, numT[:, t * P : (t + 1) * P], ident[:FP, :FP])
                recip = work.tile([P, 1], f32, tag="rc", name="recip")
                nc.vector.reciprocal(recip, pso[:, 0:1])
                nc.vector.scalar_tensor_tensor(
                    o_sb[:, t, :], pso[:, 1:FP], recip, bias_bc,
                    mybir.AluOpType.mult, mybir.AluOpType.add,
                )
            dst = out_d[I * IMW : (I + 1) * IMW, :].rearrange(
                "(a p) f -> p a f", p=P
            )
            nc.sync.dma_start(dst, o_sb)

    nc.compile()
    return nc


def _get_nc(adj_bf16: bool = False):
    key = ("nc", adj_bf16)
    if key not in _CACHE:
        _CACHE[key] = _build(adj_bf16)
    return _CACHE[key]


def kernel(x, adj, W, a, bias, adj_bf16: bool = False):
    from concourse import bass_utils

    nc = _get_nc(adj_bf16)
    in_maps = [
        {
            "x": np.ascontiguousarray(x[b], dtype=np.float32),
            "adj": np.ascontiguousarray(adj[b], dtype=np.float32),
            "W": np.ascontiguousarray(W, dtype=np.float32),
            "a": np.ascontiguousarray(a, dtype=np.float32),
            "bias": np.ascontiguousarray(bias, dtype=np.float32),
        }
        for b in range(B)
    ]
    res = bass_utils.run_bass_kernel_spmd(nc, in_maps, core_ids=list(range(B)))
    return np.stack([res.results[b]["out"] for b in range(B)]).astype(np.float32)
